# revision 72
# baseline (speedup 1.0000x reference)
"""Trainium2 Bass kernel for qk-layernorm attention (dense transformer block).

Sharding: 8 cores = 2 batches x 4 head-groups (4 heads each).  Each core
computes qkv projection (its heads only), qk-layernorm, attention, and a
partial output projection for its head slice; the host sums the 4 partials
per batch and adds b_proj.

v2 (bf16, ~280us vs 533us fp32r baseline): all matmuls run in bf16 with
fp32 PSUM accumulation - fp32 matmuls on TRN2 run at half rate
(fp32_mode=HIGH).  Design notes:
 - all matmul operands bf16 (weights cast + pre-laid-out to their SBUF
   images on host so every DMA is contiguous; first xT chunk and wqk split
   across the three HWDGE queues so compute starts ~14us in)
 - v projected directly in [m, d] layout (x tile stationary) - no PE
   transposes; a ones column per head makes each PV matmul also emit the
   softmax denominator at PSUM partition 64 for free
 - attention inner loop is software-pipelined per (mt, nq): S (row-group
   paired, concurrent) -> ACT exp (the critical engine, ~1us per [128,1024]
   tile) -> PV, with psS double-buffered; v-projection and the previous
   n-half's output-projection tiles drip into the loop's PE slack
 - LN rsqrt via ACT Sqrt + DVE reciprocal_approx_fast (exp's table set is
   loaded exactly once; Sqrt/Square/Copy share phase 1's set); softmax
   1/den via matmul partition-broadcast + reciprocal_approx_fast
 - steady state is jointly PE/ACT-saturated at ~1us per (mt, nq); the PE
   runs mostly at 1.2 GHz (chip-level SW power throttle with 8 busy cores)
 - pitfalls hit: PE column-group tiling does NOT stream concurrently
   (row groups do); accumulation groups spanning two tile_positions hang
   the hardware; ACT Ln/Exp alternation thrashes 2.7us table loads
"""

import numpy as np
import ml_dtypes

DIM = 1024
HEADS = 16
HD = 64
B = 2
N = 2048
EPS = 1e-6
N_CORES = 8
HEADS_PER_CORE = 4
PAIRS = 2          # head pairs per core
CC = 8             # contraction chunks of 128 over DIM
NT = N // 128      # 16 n/m tiles
NCH = N // 512     # 4 chunks of 512
SCALE = HD ** -0.5

BF16 = ml_dtypes.bfloat16

_prog_cache = {}


def _build_program():
    import concourse.bass as bass
    import concourse.tile as tile
    from concourse import mybir, bacc

    F32 = mybir.dt.float32
    BF = mybir.dt.bfloat16
    Act = mybir.ActivationFunctionType
    Alu = mybir.AluOpType

    nc = bacc.Bacc("TRN2", target_bir_lowering=False, debug=False,
                   num_devices=N_CORES)

    # ---- DRAM I/O ----
    # xT/weights arrive pre-rearranged to their exact SBUF images so every
    # load is a contiguous full-bandwidth DMA (>=8KB per partition row)
    xT_d = nc.dram_tensor("xT", [NCH * 128, CC * 512], BF,
                          kind="ExternalInput").ap()
    wqk_d = nc.dram_tensor("wqk", [128, CC * 512], BF, kind="ExternalInput").ap()
    wv_d = nc.dram_tensor("wv", [128, CC * 256], BF, kind="ExternalInput").ap()
    wp_d = nc.dram_tensor("wp", [128, 2 * DIM], BF, kind="ExternalInput").ap()
    smu_q_d = nc.dram_tensor("smu_q", [128, 128], BF, kind="ExternalInput").ap()
    ssq_q_d = nc.dram_tensor("ssq_q", [128, 128], BF, kind="ExternalInput").ap()
    smu_k_d = nc.dram_tensor("smu_k", [128, 128], BF, kind="ExternalInput").ap()
    ssq_k_d = nc.dram_tensor("ssq_k", [128, 128], BF, kind="ExternalInput").ap()
    selc_d = nc.dram_tensor("selc", [128, 64], BF, kind="ExternalInput").ap()
    gq_d = nc.dram_tensor("gq", [128, 1], F32, kind="ExternalInput").ap()
    gk_d = nc.dram_tensor("gk", [128, 1], F32, kind="ExternalInput").ap()
    dzer_d = nc.dram_tensor("dzer", [128, 1024], BF, kind="ExternalInput").ap()
    y_d = nc.dram_tensor("y", [N, DIM], F32, kind="ExternalOutput").ap()

    with tile.TileContext(nc) as tc:
        with tc.tile_pool(name="wts", bufs=1) as wts, \
             tc.tile_pool(name="persist", bufs=1) as persist:
            # ---- persistent SBUF tensors ----
            # weights go on the Activation HWDGE queue so the first xT
            # chunk (SP queue, issued in phase 1) streams immediately
            wqk = wts.tile([128, CC * 512], BF)         # [c, cc x (q256|k256)]
            wv = wts.tile([128, CC * 256], BF)          # [c, cc x 256]
            wp = wts.tile([128, 2 * DIM], BF)           # [d-pair, pc x 1024]
            # halves, so the first job matmuls start after half a transfer
            nc.scalar.dma_start(wqk[:, 0:2048], wqk_d[:, 0:2048])
            nc.scalar.dma_start(wqk[:, 2048:4096], wqk_d[:, 2048:4096])
            smu = [wts.tile([128, 128], BF, tag=f"smu{t}", name=f"smu{t}")
                   for t in range(2)]
            ssq = [wts.tile([128, 128], BF, tag=f"ssq{t}", name=f"ssq{t}")
                   for t in range(2)]
            nc.scalar.dma_start(smu[0][:], smu_q_d[:])
            nc.scalar.dma_start(ssq[0][:], ssq_q_d[:])
            nc.scalar.dma_start(smu[1][:], smu_k_d[:])
            nc.scalar.dma_start(ssq[1][:], ssq_k_d[:])
            gcol = [wts.tile([128, 1], F32, tag=f"g{t}", name=f"g{t}")
                    for t in range(2)]
            nc.scalar.dma_start(gcol[0][:], gq_d[:])
            nc.scalar.dma_start(gcol[1][:], gk_d[:])
            nc.scalar.dma_start(wv[:], wv_d[:])
            nc.scalar.dma_start(wp[:], wp_d[:])
            selc = wts.tile([128, 64], BF)    # row 64 -> 64 out partitions
            nc.scalar.dma_start(selc[:], selc_d[:])
            epsb = wts.tile([128, 1], F32)
            nc.gpsimd.memset(epsb[:], EPS)

            # full xT resident in SBUF, chunk-major: [c-part, nch, cc, 512]
            xt = persist.tile([128, CC * N], BF)

            # qk[0],qk[1]: q pair tiles; qk[2],qk[3]: k pair tiles [d-pair, n]
            qk = [persist.tile([128, N], BF, tag=f"qk{i}", name=f"qk{i}")
                  for i in range(4)]
            # v: [m, mt x (4 heads x [64 v | 1 ones])] - the ones column
            # makes each PV matmul emit the softmax denominator at psum
            # partition 64 for free.  (memset is issued later so it doesn't
            # delay the gpsimd DMA queue at startup)
            v_sb = persist.tile([128, NT * 260], BF)
            onorm = [persist.tile([128, N], BF, tag=f"on{p}", name=f"on{p}")
                     for p in range(PAIRS)]
            den_sbh = [wts.tile([128, 1024], BF, tag=f"dsb{h}", name=f"dsb{h}")
                       for h in range(2)]
            nc.scalar.dma_start(den_sbh[0][:], dzer_d[:])
            nc.scalar.dma_start(den_sbh[1][:], dzer_d[:])

            # ============ Phase 1: qk projection + qk-layernorm ============
            with tc.tile_pool(name="p1s", bufs=2) as p1s, \
                 tc.tile_pool(name="ps1", bufs=1, space="PSUM") as ps1:
                for nch in range(NCH):
                    sl = slice(nch * 512, (nch + 1) * 512)
                    if nch == 0:
                        # chunk 0 split across two queues: compute can start
                        # after the first cc-half lands
                        nc.sync.dma_start(xt[:, 0:2048], xT_d[0:128, 0:2048])
                        nc.gpsimd.dma_start(xt[:, 2048:4096],
                                            xT_d[0:128, 2048:4096])
                        nc.gpsimd.memset(v_sb[:], 1.0)
                    else:
                        nc.sync.dma_start(xt[:, nch * 4096:(nch + 1) * 4096],
                                          xT_d[nch * 128:(nch + 1) * 128, :])
                    # two sub-blocks: (q pair_p, k pair_p) for p = 0, 1
                    for p in range(PAIRS):
                        accq = ps1.tile([128, 512], F32, tag="accq", bufs=2,
                                        name="accq")
                        acck = ps1.tile([128, 512], F32, tag="acck", bufs=2,
                                        name="acck")
                        for cc in range(CC):
                            nc.tensor.matmul(
                                accq[:], wqk[:, cc * 512 + p * 128:
                                             cc * 512 + p * 128 + 128],
                                xt[:, nch * 4096 + cc * 512:
                                   nch * 4096 + cc * 512 + 512],
                                start=(cc == 0), stop=(cc == CC - 1))
                        for cc in range(CC):
                            nc.tensor.matmul(
                                acck[:], wqk[:, cc * 512 + 256 + p * 128:
                                             cc * 512 + 256 + p * 128 + 128],
                                xt[:, nch * 4096 + cc * 512:
                                   nch * 4096 + cc * 512 + 512],
                                start=(cc == 0), stop=(cc == CC - 1))
                        # qk-layernorm for both tiles of this pair.
                        # Engine split: qb/mu2/sqrt on ACT (all in the sqrt
                        # table set - exp's set is only loaded once, for
                        # phase 3), squares of qb on GPSIMD (SBUF-only), rest
                        # on DVE.
                        for t, acc in ((0, accq), (1, acck)):
                            dest = qk[2 * t + p]
                            qb = p1s.tile([128, 512], BF, tag="qb", name="qb")
                            nc.scalar.copy(qb[:], acc[:])
                            sqc = p1s.tile([128, 512], BF, tag="sqc", name="sqc")
                            nc.gpsimd.tensor_mul(sqc[:], qb[:], qb[:])
                            pmu = ps1.tile([128, 512], F32, tag="pmu", bufs=2,
                                           name="pmu")
                            psq = ps1.tile([128, 512], F32, tag="psq", bufs=2,
                                           name="psq")
                            nc.tensor.matmul(pmu[:], smu[t][:], qb[:],
                                             start=True, stop=True)
                            nc.tensor.matmul(psq[:], ssq[t][:], sqc[:],
                                             start=True, stop=True)
                            mu2 = p1s.tile([128, 512], F32, tag="mu2", name="mu2")
                            nc.scalar.activation(mu2[:], pmu[:], Act.Square)
                            var = p1s.tile([128, 512], F32, tag="var", name="var")
                            nc.vector.tensor_sub(var[:], psq[:], mu2[:])
                            sd = p1s.tile([128, 512], F32, tag="sd", name="sd")
                            nc.scalar.activation(sd[:], var[:], Act.Sqrt,
                                                 bias=epsb[:])
                            rs = p1s.tile([128, 512], F32, tag="rs", name="rs")
                            nc.vector.reciprocal_approx_fast(rs[:], sd[:])
                            t1 = p1s.tile([128, 512], F32, tag="t1", name="t1")
                            nc.vector.scalar_tensor_tensor(
                                t1[:], pmu[:], gcol[t][:], qb[:],
                                op0=Alu.mult, op1=Alu.subtract)
                            nc.vector.scalar_tensor_tensor(
                                dest[:, sl], t1[:], -1.0, rs[:],
                                op0=Alu.mult, op1=Alu.mult)

            # ================= Phase 3: attention =================
            # (v projection is dripped into the first pair group's pipeline)
            # PSUM: psS 2 banks x 2 bufs + poh 2 banks x 2 heads = 8; the
            # pb/py tiles recycle the psS tag slots between pair groups.
            with tc.tile_pool(name="p3", bufs=2) as p3, \
                 tc.tile_pool(name="ps3", bufs=1, space="PSUM") as ps3:

                def emit_v(mt):
                    vt = ps3.tile([128, 1024], F32, tag="psS", bufs=2,
                                  name="vt")
                    for cc in range(CC):
                        base = (mt // 4) * 4096 + cc * 512 + (mt % 4) * 128
                        nc.tensor.matmul(
                            vt[:, 0:256], xt[:, base:base + 128],
                            wv[:, cc * 256:(cc + 1) * 256],
                            start=(cc == 0), stop=(cc == CC - 1))
                    nc.vector.tensor_copy(
                        v_sb[:, mt * 260:(mt + 1) * 260].rearrange(
                            "p (hl c) -> p hl c", c=65)[:, :, 0:64],
                        vt[:, 0:256].rearrange("p (hl c) -> p hl c", c=64))

                def emit_proj(nt, flush=False):
                    py = ps3.tile([128, 1024], F32, tag="psS", bufs=2,
                                  name="py")
                    for oc in range(2):
                        for pp in range(PAIRS):
                            nc.tensor.matmul(
                                py[:, oc * 512:(oc + 1) * 512],
                                onorm[pp][:, nt * 128:(nt + 1) * 128],
                                wp[:, pp * 1024 + oc * 512:
                                   pp * 1024 + oc * 512 + 512],
                                start=(pp == 0), stop=(pp == PAIRS - 1))
                    yt = p3.tile([128, 1024], F32, tag="yt", bufs=3,
                                 name="yt")
                    if flush and nt % 2 == 0:
                        # in the final flush ACT is idle; alternating the
                        # copies keeps the py rotation from stalling
                        nc.scalar.copy(yt[:], py[:])
                    else:
                        nc.vector.tensor_copy(yt[:], py[:])
                    nc.sync.dma_start(y_d[nt * 128:(nt + 1) * 128, :], yt[:])

                pending_proj = []
                for nh in range(2):            # halves of n (1024 each)
                    for p in range(PAIRS):
                        qt, kt = qk[p], qk[2 + p]
                        # per-head o^T accumulators [d 0-63 | den at 64]
                        poh = [ps3.tile([128, 1024], F32, tag=f"poh{h}",
                                        bufs=1, name=f"poh{h}")
                               for h in range(2)]

                        def emit_pv(mt, eSs):
                            for h in range(2):
                                for nq in range(2):
                                    nc.tensor.matmul(
                                        poh[h][0:65, nq * 512:(nq + 1) * 512],
                                        v_sb[:, mt * 260 + (2 * p + h) * 65:
                                             mt * 260 + (2 * p + h) * 65 + 65],
                                        eSs[nq][:, h * 512:(h + 1) * 512],
                                        start=(mt == 0), stop=(mt == NT - 1))

                        # software pipeline: S(mt)/exp(mt) issued one mt
                        # ahead of PV(mt-1) so the PE never waits on exp
                        if nh == 0 and p == 0:
                            for vm in range(3):     # v prologue
                                emit_v(vm)
                        prev = None
                        for mt in range(NT):
                            # v tiles for the first group and the previous
                            # n-half's projection tiles drip into the loop
                            # so their chains overlap the pipeline
                            if nh == 0 and p == 0 and mt < 13:
                                emit_v(mt + 3)
                            if pending_proj and mt % 4 == 1:
                                emit_proj(pending_proj.pop(0))
                            eSs = []
                            for nq in range(2):
                                psS = ps3.tile([128, 1024], F32, tag="psS",
                                               bufs=2, name="psS")
                                for h in range(2):   # row-group pairs
                                    hs = slice(h * 64, (h + 1) * 64)
                                    nc.tensor.matmul(
                                        psS[:, h * 512:(h + 1) * 512],
                                        kt[hs, mt * 128:(mt + 1) * 128],
                                        qt[hs, nh * 1024 + nq * 512:
                                           nh * 1024 + nq * 512 + 512],
                                        start=True, stop=True)
                                eS = p3.tile([128, 1024], BF, tag="eS",
                                             bufs=5, name="eS")
                                nc.scalar.activation(eS[:], psS[:], Act.Exp,
                                                     scale=float(SCALE))
                                eSs.append(eS)
                            if prev is not None:
                                emit_pv(*prev)
                            prev = (mt, eSs)
                        emit_pv(*prev)
                        # 1/den -> broadcast -> normalize
                        for h in range(2):
                            nc.vector.tensor_copy(den_sbh[h][64:65, :],
                                                  poh[h][64:65, :])
                        pb = [ps3.tile([128, 1024], F32, tag="psS", bufs=2,
                                       name=f"pb{h}") for h in range(2)]
                        for h in range(2):
                            for nq in range(2):
                                nc.tensor.matmul(
                                    pb[h][0:64, nq * 512:(nq + 1) * 512],
                                    selc[:],
                                    den_sbh[h][:, nq * 512:(nq + 1) * 512],
                                    start=True, stop=True)
                        rds = []
                        for h in range(2):
                            rd = p3.tile([128, 1024], F32, tag="rd",
                                         name=f"rd{h}")
                            nc.vector.reciprocal_approx_fast(rd[0:64, :],
                                                             pb[h][0:64, :])
                            rds.append(rd)
                        nc.vector.tensor_mul(
                            onorm[p][0:64, nh * 1024:(nh + 1) * 1024],
                            poh[0][0:64, :], rds[0][0:64, :])
                        tmpB = p3.tile([128, 1024], BF, tag="tmpB",
                                       name="tmpB")
                        nc.vector.tensor_mul(tmpB[0:64, :], poh[1][0:64, :],
                                             rds[1][0:64, :])
                        nc.sync.dma_start(
                            onorm[p][64:128, nh * 1024:(nh + 1) * 1024],
                            tmpB[0:64, :])
                    # projection for this n-half: deferred into the next
                    # group's pipeline (flushed below for the last one)
                    pending_proj.extend(range(nh * 8, (nh + 1) * 8))
                for nt in pending_proj:
                    emit_proj(nt, flush=True)

    nc.compile()
    return nc


def _prep_core_inputs(x, W_qkv, q_gamma, k_gamma, W_proj):
    """Host-side sharding + layout prep. Returns list of 8 in_maps."""
    f32 = np.float32
    blkdiag = np.kron(np.eye(2, dtype=f32), np.ones((64, 64), f32))
    # selc: broadcast den (psum partition 64) to 64 output partitions
    selc = np.zeros((128, 64), f32)
    selc[64, :] = 1.0
    in_maps = []
    for core in range(N_CORES):
        b, g = core // 4, core % 4
        heads = [4 * g + j for j in range(HEADS_PER_CORE)]
        qcols = np.concatenate(
            [(W_qkv[h * HD:(h + 1) * HD, :] * q_gamma[:, None]).T for h in heads],
            axis=1)
        kcols = np.concatenate(
            [(W_qkv[DIM + h * HD:DIM + (h + 1) * HD, :] * k_gamma[:, None]).T
             for h in heads], axis=1)
        wqk = np.concatenate([qcols, kcols], axis=1)            # [1024, 512]
        wv = np.concatenate(
            [W_qkv[2 * DIM + h * HD:2 * DIM + (h + 1) * HD, :].T
             for h in heads], axis=1)                           # [1024, 256]
        wp = W_proj[:, heads[0] * HD:(heads[-1] + 1) * HD].T    # [256, 1024]
        # pre-rearrange to the exact SBUF images (contiguous DMA lines)
        wqk = wqk.reshape(CC, 128, 512).transpose(1, 0, 2).reshape(128, -1)
        wv = wv.reshape(CC, 128, 256).transpose(1, 0, 2).reshape(128, -1)
        wp = wp.reshape(2, 128, DIM).transpose(1, 0, 2).reshape(128, -1)
        xT = x[b].T.reshape(CC, 128, NCH, 512).transpose(2, 1, 0, 3)
        g2q = np.tile(q_gamma, 2).astype(f32)
        g2k = np.tile(k_gamma, 2).astype(f32)
        in_maps.append({
            "xT": np.ascontiguousarray(xT).reshape(NCH * 128, CC * 512).astype(BF16),
            "wqk": np.ascontiguousarray(wqk).astype(BF16),
            "wv": np.ascontiguousarray(wv).astype(BF16),
            "wp": np.ascontiguousarray(wp).astype(BF16),
            "smu_q": (blkdiag * (1.0 / (64.0 * g2q))[:, None]).astype(BF16),
            "ssq_q": (blkdiag * (1.0 / (64.0 * g2q * g2q))[:, None]).astype(BF16),
            "smu_k": (blkdiag * (1.0 / (64.0 * g2k))[:, None]).astype(BF16),
            "ssq_k": (blkdiag * (1.0 / (64.0 * g2k * g2k))[:, None]).astype(BF16),
            "selc": selc.astype(BF16),
            "gq": g2q[:, None].copy(), "gk": g2k[:, None].copy(),
            "dzer": np.zeros((128, 1024), f32).astype(BF16),
        })
    return in_maps


def _numpy_fallback(x, W_qkv, q_gamma, q_beta, k_gamma, k_beta, W_proj, b_proj):
    def ln(t, gamma, beta):
        mu = t.mean(-1, keepdims=True)
        var = ((t - mu) ** 2).mean(-1, keepdims=True)
        return (t - mu) / np.sqrt(var + EPS) * gamma + beta
    Bs, Ns, C = x.shape
    qkv = np.einsum('bnc,oc->bno', x, W_qkv)
    qkv = qkv.reshape(Bs, Ns, 3, HEADS, HD).transpose(2, 0, 3, 1, 4)
    q, k, v = ln(qkv[0], q_gamma, q_beta), ln(qkv[1], k_gamma, k_beta), qkv[2]
    s = np.einsum('bhnd,bhmd->bhnm', q * SCALE, k)
    s = np.exp(s - s.max(-1, keepdims=True))
    p = s / s.sum(-1, keepdims=True)
    o = np.einsum('bhnm,bhmd->bhnd', p, v)
    o = o.transpose(0, 2, 1, 3).reshape(Bs, Ns, C)
    return (np.einsum('bnc,oc->bno', o, W_proj) + b_proj).astype(np.float32)


def kernel(x, W_qkv, q_gamma, q_beta, k_gamma, k_beta, W_proj, b_proj):
    x = np.asarray(x, np.float32)
    W_qkv = np.asarray(W_qkv, np.float32)
    q_gamma = np.asarray(q_gamma, np.float32)
    q_beta = np.asarray(q_beta, np.float32)
    k_gamma = np.asarray(k_gamma, np.float32)
    k_beta = np.asarray(k_beta, np.float32)
    W_proj = np.asarray(W_proj, np.float32)
    b_proj = np.asarray(b_proj, np.float32)

    if np.any(q_beta != 0) or np.any(k_beta != 0):
        # beta terms are not wired into the device kernel (reference always
        # uses beta = 0); fall back to exact host computation
        return _numpy_fallback(x, W_qkv, q_gamma, q_beta, k_gamma, k_beta,
                               W_proj, b_proj)

    from concourse import bass_utils

    if "prog" not in _prog_cache:
        _prog_cache["prog"] = _build_program()
    nc = _prog_cache["prog"]

    in_maps = _prep_core_inputs(x, W_qkv, q_gamma, k_gamma, W_proj)
    res = bass_utils.run_bass_kernel_spmd(nc, in_maps, list(range(N_CORES)))

    out = np.empty((B, N, DIM), np.float32)
    for b in range(B):
        acc = res.results[4 * b + 0]["y"].astype(np.float32).copy()
        for g in range(1, 4):
            acc += res.results[4 * b + g]["y"]
        out[b] = acc + b_proj
    return out


# revision 74
# speedup vs baseline: 1.0029x; 1.0029x over previous
"""Trainium2 Bass kernel for qk-layernorm attention (dense transformer block).

Sharding: 8 cores = 2 batches x 4 head-groups (4 heads each).  Each core
computes qkv projection (its heads only), qk-layernorm, attention, and a
partial output projection for its head slice; the host sums the 4 partials
per batch and adds b_proj.

v2 (bf16, ~280us vs 533us fp32r baseline): all matmuls run in bf16 with
fp32 PSUM accumulation - fp32 matmuls on TRN2 run at half rate
(fp32_mode=HIGH).  Design notes:
 - all matmul operands bf16 (weights cast + pre-laid-out to their SBUF
   images on host so every DMA is contiguous; first xT chunk and wqk split
   across the three HWDGE queues so compute starts ~14us in)
 - v projected directly in [m, d] layout (x tile stationary) - no PE
   transposes; a ones column per head makes each PV matmul also emit the
   softmax denominator at PSUM partition 64 for free
 - attention inner loop is software-pipelined per (mt, nq): S (row-group
   paired, concurrent) -> ACT exp (the critical engine, ~1us per [128,1024]
   tile) -> PV, with psS double-buffered; v-projection and the previous
   n-half's output-projection tiles drip into the loop's PE slack
 - LN rsqrt via ACT Sqrt + DVE reciprocal_approx_fast (exp's table set is
   loaded exactly once; Sqrt/Square/Copy share phase 1's set); softmax
   1/den via matmul partition-broadcast + reciprocal_approx_fast
 - steady state is jointly PE/ACT-saturated at ~1us per (mt, nq); the PE
   runs mostly at 1.2 GHz (chip-level SW power throttle with 8 busy cores)
 - pitfalls hit: PE column-group tiling does NOT stream concurrently
   (row groups do); accumulation groups spanning two tile_positions hang
   the hardware; ACT Ln/Exp alternation thrashes 2.7us table loads
"""

import numpy as np
import ml_dtypes

DIM = 1024
HEADS = 16
HD = 64
B = 2
N = 2048
EPS = 1e-6
N_CORES = 8
HEADS_PER_CORE = 4
PAIRS = 2          # head pairs per core
CC = 8             # contraction chunks of 128 over DIM
NT = N // 128      # 16 n/m tiles
NCH = N // 512     # 4 chunks of 512
SCALE = HD ** -0.5

BF16 = ml_dtypes.bfloat16

_prog_cache = {}


def _build_program():
    import concourse.bass as bass
    import concourse.tile as tile
    from concourse import mybir, bacc

    F32 = mybir.dt.float32
    BF = mybir.dt.bfloat16
    Act = mybir.ActivationFunctionType
    Alu = mybir.AluOpType

    nc = bacc.Bacc("TRN2", target_bir_lowering=False, debug=False,
                   num_devices=N_CORES)

    # ---- DRAM I/O ----
    # xT/weights arrive pre-rearranged to their exact SBUF images so every
    # load is a contiguous full-bandwidth DMA (>=8KB per partition row)
    xT_d = nc.dram_tensor("xT", [NCH * 128, CC * 512], BF,
                          kind="ExternalInput").ap()
    wqk_d = nc.dram_tensor("wqk", [128, CC * 512], BF, kind="ExternalInput").ap()
    wv_d = nc.dram_tensor("wv", [128, CC * 256], BF, kind="ExternalInput").ap()
    wp_d = nc.dram_tensor("wp", [128, 2 * DIM], BF, kind="ExternalInput").ap()
    smu_q_d = nc.dram_tensor("smu_q", [128, 128], BF, kind="ExternalInput").ap()
    ssq_q_d = nc.dram_tensor("ssq_q", [128, 128], BF, kind="ExternalInput").ap()
    smu_k_d = nc.dram_tensor("smu_k", [128, 128], BF, kind="ExternalInput").ap()
    ssq_k_d = nc.dram_tensor("ssq_k", [128, 128], BF, kind="ExternalInput").ap()
    selc_d = nc.dram_tensor("selc", [128, 64], BF, kind="ExternalInput").ap()
    gq_d = nc.dram_tensor("gq", [128, 1], F32, kind="ExternalInput").ap()
    gk_d = nc.dram_tensor("gk", [128, 1], F32, kind="ExternalInput").ap()
    dzer_d = nc.dram_tensor("dzer", [128, 1024], BF, kind="ExternalInput").ap()
    y_d = nc.dram_tensor("y", [N, DIM], F32, kind="ExternalOutput").ap()

    with tile.TileContext(nc) as tc:
        with tc.tile_pool(name="wts", bufs=1) as wts, \
             tc.tile_pool(name="persist", bufs=1) as persist:
            # ---- persistent SBUF tensors ----
            # weights go on the Activation HWDGE queue so the first xT
            # chunk (SP queue, issued in phase 1) streams immediately
            wqk = wts.tile([128, CC * 512], BF)         # [c, cc x (q256|k256)]
            wv = wts.tile([128, CC * 256], BF)          # [c, cc x 256]
            wp = wts.tile([128, 2 * DIM], BF)           # [d-pair, pc x 1024]
            # quarters, so the first job matmuls start after ~1/4 transfer
            for qtr in range(4):
                nc.scalar.dma_start(wqk[:, qtr * 1024:(qtr + 1) * 1024],
                                    wqk_d[:, qtr * 1024:(qtr + 1) * 1024])
            smu = [wts.tile([128, 128], BF, tag=f"smu{t}", name=f"smu{t}")
                   for t in range(2)]
            ssq = [wts.tile([128, 128], BF, tag=f"ssq{t}", name=f"ssq{t}")
                   for t in range(2)]
            nc.scalar.dma_start(smu[0][:], smu_q_d[:])
            nc.scalar.dma_start(ssq[0][:], ssq_q_d[:])
            nc.scalar.dma_start(smu[1][:], smu_k_d[:])
            nc.scalar.dma_start(ssq[1][:], ssq_k_d[:])
            gcol = [wts.tile([128, 1], F32, tag=f"g{t}", name=f"g{t}")
                    for t in range(2)]
            nc.scalar.dma_start(gcol[0][:], gq_d[:])
            nc.scalar.dma_start(gcol[1][:], gk_d[:])
            nc.scalar.dma_start(wv[:], wv_d[:])
            nc.scalar.dma_start(wp[:], wp_d[:])
            selc = wts.tile([128, 64], BF)    # row 64 -> 64 out partitions
            nc.scalar.dma_start(selc[:], selc_d[:])
            epsb = wts.tile([128, 1], F32)
            nc.gpsimd.memset(epsb[:], EPS)

            # full xT resident in SBUF, chunk-major: [c-part, nch, cc, 512]
            xt = persist.tile([128, CC * N], BF)

            # qk[0],qk[1]: q pair tiles; qk[2],qk[3]: k pair tiles [d-pair, n]
            qk = [persist.tile([128, N], BF, tag=f"qk{i}", name=f"qk{i}")
                  for i in range(4)]
            # v: [m, mt x (4 heads x [64 v | 1 ones])] - the ones column
            # makes each PV matmul emit the softmax denominator at psum
            # partition 64 for free.  (memset is issued later so it doesn't
            # delay the gpsimd DMA queue at startup)
            v_sb = persist.tile([128, NT * 260], BF)
            onorm = [persist.tile([128, N], BF, tag=f"on{p}", name=f"on{p}")
                     for p in range(PAIRS)]
            den_sbh = [wts.tile([128, 1024], BF, tag=f"dsb{h}", name=f"dsb{h}")
                       for h in range(2)]
            nc.scalar.dma_start(den_sbh[0][:], dzer_d[:])
            nc.scalar.dma_start(den_sbh[1][:], dzer_d[:])

            # ============ Phase 1: qk projection + qk-layernorm ============
            with tc.tile_pool(name="p1s", bufs=2) as p1s, \
                 tc.tile_pool(name="ps1", bufs=1, space="PSUM") as ps1:
                for nch in range(NCH):
                    sl = slice(nch * 512, (nch + 1) * 512)
                    if nch == 0:
                        # chunk 0 split across two queues: compute can start
                        # after the first cc-half lands
                        nc.sync.dma_start(xt[:, 0:2048], xT_d[0:128, 0:2048])
                        nc.gpsimd.dma_start(xt[:, 2048:4096],
                                            xT_d[0:128, 2048:4096])
                        nc.gpsimd.memset(v_sb[:], 1.0)
                    else:
                        nc.sync.dma_start(xt[:, nch * 4096:(nch + 1) * 4096],
                                          xT_d[nch * 128:(nch + 1) * 128, :])
                    # two sub-blocks: (q pair_p, k pair_p) for p = 0, 1
                    for p in range(PAIRS):
                        accq = ps1.tile([128, 512], F32, tag="accq", bufs=2,
                                        name="accq")
                        acck = ps1.tile([128, 512], F32, tag="acck", bufs=2,
                                        name="acck")
                        # q/k interleaved per cc so early weight quarters
                        # are fully consumed in DMA arrival order
                        for cc in range(CC):
                            mv = slice(nch * 4096 + cc * 512,
                                       nch * 4096 + cc * 512 + 512)
                            nc.tensor.matmul(
                                accq[:], wqk[:, cc * 512 + p * 128:
                                             cc * 512 + p * 128 + 128],
                                xt[:, mv],
                                start=(cc == 0), stop=(cc == CC - 1))
                            nc.tensor.matmul(
                                acck[:], wqk[:, cc * 512 + 256 + p * 128:
                                             cc * 512 + 256 + p * 128 + 128],
                                xt[:, mv],
                                start=(cc == 0), stop=(cc == CC - 1))
                        # qk-layernorm for both tiles of this pair.
                        # Engine split: qb/mu2/sqrt on ACT (all in the sqrt
                        # table set - exp's set is only loaded once, for
                        # phase 3), squares of qb on GPSIMD (SBUF-only), rest
                        # on DVE.
                        for t, acc in ((0, accq), (1, acck)):
                            dest = qk[2 * t + p]
                            qb = p1s.tile([128, 512], BF, tag="qb", name="qb")
                            nc.scalar.copy(qb[:], acc[:])
                            sqc = p1s.tile([128, 512], BF, tag="sqc", name="sqc")
                            nc.gpsimd.tensor_mul(sqc[:], qb[:], qb[:])
                            pmu = ps1.tile([128, 512], F32, tag="pmu", bufs=2,
                                           name="pmu")
                            psq = ps1.tile([128, 512], F32, tag="psq", bufs=2,
                                           name="psq")
                            nc.tensor.matmul(pmu[:], smu[t][:], qb[:],
                                             start=True, stop=True)
                            nc.tensor.matmul(psq[:], ssq[t][:], sqc[:],
                                             start=True, stop=True)
                            mu2 = p1s.tile([128, 512], F32, tag="mu2", name="mu2")
                            nc.scalar.activation(mu2[:], pmu[:], Act.Square)
                            var = p1s.tile([128, 512], F32, tag="var", name="var")
                            nc.vector.tensor_sub(var[:], psq[:], mu2[:])
                            sd = p1s.tile([128, 512], F32, tag="sd", name="sd")
                            nc.scalar.activation(sd[:], var[:], Act.Sqrt,
                                                 bias=epsb[:])
                            rs = p1s.tile([128, 512], F32, tag="rs", name="rs")
                            nc.vector.reciprocal_approx_fast(rs[:], sd[:])
                            t1 = p1s.tile([128, 512], F32, tag="t1", name="t1")
                            nc.vector.scalar_tensor_tensor(
                                t1[:], pmu[:], gcol[t][:], qb[:],
                                op0=Alu.mult, op1=Alu.subtract)
                            nc.vector.scalar_tensor_tensor(
                                dest[:, sl], t1[:], -1.0, rs[:],
                                op0=Alu.mult, op1=Alu.mult)

            # ================= Phase 3: attention =================
            # (v projection is dripped into the first pair group's pipeline)
            # PSUM: psS 2 banks x 2 bufs + poh 2 banks x 2 heads = 8; the
            # pb/py tiles recycle the psS tag slots between pair groups.
            with tc.tile_pool(name="p3", bufs=2) as p3, \
                 tc.tile_pool(name="ps3", bufs=1, space="PSUM") as ps3:

                def emit_v(mt):
                    vt = ps3.tile([128, 1024], F32, tag="psS", bufs=2,
                                  name="vt")
                    for cc in range(CC):
                        base = (mt // 4) * 4096 + cc * 512 + (mt % 4) * 128
                        nc.tensor.matmul(
                            vt[:, 0:256], xt[:, base:base + 128],
                            wv[:, cc * 256:(cc + 1) * 256],
                            start=(cc == 0), stop=(cc == CC - 1))
                    nc.vector.tensor_copy(
                        v_sb[:, mt * 260:(mt + 1) * 260].rearrange(
                            "p (hl c) -> p hl c", c=65)[:, :, 0:64],
                        vt[:, 0:256].rearrange("p (hl c) -> p hl c", c=64))

                def emit_proj(nt, flush=False):
                    py = ps3.tile([128, 1024], F32, tag="psS", bufs=2,
                                  name="py")
                    for oc in range(2):
                        for pp in range(PAIRS):
                            nc.tensor.matmul(
                                py[:, oc * 512:(oc + 1) * 512],
                                onorm[pp][:, nt * 128:(nt + 1) * 128],
                                wp[:, pp * 1024 + oc * 512:
                                   pp * 1024 + oc * 512 + 512],
                                start=(pp == 0), stop=(pp == PAIRS - 1))
                    yt = p3.tile([128, 1024], F32, tag="yt", bufs=3,
                                 name="yt")
                    if flush and nt % 2 == 0:
                        # in the final flush ACT is idle; alternating the
                        # copies keeps the py rotation from stalling
                        nc.scalar.copy(yt[:], py[:])
                    else:
                        nc.vector.tensor_copy(yt[:], py[:])
                    nc.sync.dma_start(y_d[nt * 128:(nt + 1) * 128, :], yt[:])

                pending_proj = []
                for nh in range(2):            # halves of n (1024 each)
                    for p in range(PAIRS):
                        qt, kt = qk[p], qk[2 + p]
                        # per-head o^T accumulators [d 0-63 | den at 64]
                        poh = [ps3.tile([128, 1024], F32, tag=f"poh{h}",
                                        bufs=1, name=f"poh{h}")
                               for h in range(2)]

                        def emit_pv(mt, eSs):
                            for h in range(2):
                                for nq in range(2):
                                    nc.tensor.matmul(
                                        poh[h][0:65, nq * 512:(nq + 1) * 512],
                                        v_sb[:, mt * 260 + (2 * p + h) * 65:
                                             mt * 260 + (2 * p + h) * 65 + 65],
                                        eSs[nq][:, h * 512:(h + 1) * 512],
                                        start=(mt == 0), stop=(mt == NT - 1))

                        # software pipeline: S(mt)/exp(mt) issued one mt
                        # ahead of PV(mt-1) so the PE never waits on exp
                        if nh == 0 and p == 0:
                            for vm in range(3):     # v prologue
                                emit_v(vm)
                        prev = None
                        for mt in range(NT):
                            # v tiles for the first group and the previous
                            # n-half's projection tiles drip into the loop
                            # so their chains overlap the pipeline
                            if nh == 0 and p == 0 and mt < 13:
                                emit_v(mt + 3)
                            if pending_proj and mt % 4 == 1:
                                emit_proj(pending_proj.pop(0))
                            eSs = []
                            for nq in range(2):
                                psS = ps3.tile([128, 1024], F32, tag="psS",
                                               bufs=2, name="psS")
                                for h in range(2):   # row-group pairs
                                    hs = slice(h * 64, (h + 1) * 64)
                                    nc.tensor.matmul(
                                        psS[:, h * 512:(h + 1) * 512],
                                        kt[hs, mt * 128:(mt + 1) * 128],
                                        qt[hs, nh * 1024 + nq * 512:
                                           nh * 1024 + nq * 512 + 512],
                                        start=True, stop=True)
                                eS = p3.tile([128, 1024], BF, tag="eS",
                                             bufs=5, name="eS")
                                nc.scalar.activation(eS[:], psS[:], Act.Exp,
                                                     scale=float(SCALE))
                                eSs.append(eS)
                            if prev is not None:
                                emit_pv(*prev)
                            prev = (mt, eSs)
                        emit_pv(*prev)
                        # 1/den -> broadcast -> normalize
                        for h in range(2):
                            nc.vector.tensor_copy(den_sbh[h][64:65, :],
                                                  poh[h][64:65, :])
                        pb = [ps3.tile([128, 1024], F32, tag="psS", bufs=2,
                                       name=f"pb{h}") for h in range(2)]
                        for h in range(2):
                            for nq in range(2):
                                nc.tensor.matmul(
                                    pb[h][0:64, nq * 512:(nq + 1) * 512],
                                    selc[:],
                                    den_sbh[h][:, nq * 512:(nq + 1) * 512],
                                    start=True, stop=True)
                        rds = []
                        for h in range(2):
                            rd = p3.tile([128, 1024], F32, tag="rd",
                                         name=f"rd{h}")
                            nc.vector.reciprocal_approx_fast(rd[0:64, :],
                                                             pb[h][0:64, :])
                            rds.append(rd)
                        nc.vector.tensor_mul(
                            onorm[p][0:64, nh * 1024:(nh + 1) * 1024],
                            poh[0][0:64, :], rds[0][0:64, :])
                        tmpB = p3.tile([128, 1024], BF, tag="tmpB",
                                       name="tmpB")
                        nc.vector.tensor_mul(tmpB[0:64, :], poh[1][0:64, :],
                                             rds[1][0:64, :])
                        nc.sync.dma_start(
                            onorm[p][64:128, nh * 1024:(nh + 1) * 1024],
                            tmpB[0:64, :])
                    # projection for this n-half: deferred into the next
                    # group's pipeline (flushed below for the last one)
                    pending_proj.extend(range(nh * 8, (nh + 1) * 8))
                for nt in pending_proj:
                    emit_proj(nt, flush=True)

    nc.compile()
    return nc


def _prep_core_inputs(x, W_qkv, q_gamma, k_gamma, W_proj):
    """Host-side sharding + layout prep. Returns list of 8 in_maps."""
    f32 = np.float32
    blkdiag = np.kron(np.eye(2, dtype=f32), np.ones((64, 64), f32))
    # selc: broadcast den (psum partition 64) to 64 output partitions
    selc = np.zeros((128, 64), f32)
    selc[64, :] = 1.0
    in_maps = []
    for core in range(N_CORES):
        b, g = core // 4, core % 4
        heads = [4 * g + j for j in range(HEADS_PER_CORE)]
        qcols = np.concatenate(
            [(W_qkv[h * HD:(h + 1) * HD, :] * q_gamma[:, None]).T for h in heads],
            axis=1)
        kcols = np.concatenate(
            [(W_qkv[DIM + h * HD:DIM + (h + 1) * HD, :] * k_gamma[:, None]).T
             for h in heads], axis=1)
        wqk = np.concatenate([qcols, kcols], axis=1)            # [1024, 512]
        wv = np.concatenate(
            [W_qkv[2 * DIM + h * HD:2 * DIM + (h + 1) * HD, :].T
             for h in heads], axis=1)                           # [1024, 256]
        wp = W_proj[:, heads[0] * HD:(heads[-1] + 1) * HD].T    # [256, 1024]
        # pre-rearrange to the exact SBUF images (contiguous DMA lines)
        wqk = wqk.reshape(CC, 128, 512).transpose(1, 0, 2).reshape(128, -1)
        wv = wv.reshape(CC, 128, 256).transpose(1, 0, 2).reshape(128, -1)
        wp = wp.reshape(2, 128, DIM).transpose(1, 0, 2).reshape(128, -1)
        xT = x[b].T.reshape(CC, 128, NCH, 512).transpose(2, 1, 0, 3)
        g2q = np.tile(q_gamma, 2).astype(f32)
        g2k = np.tile(k_gamma, 2).astype(f32)
        in_maps.append({
            "xT": np.ascontiguousarray(xT).reshape(NCH * 128, CC * 512).astype(BF16),
            "wqk": np.ascontiguousarray(wqk).astype(BF16),
            "wv": np.ascontiguousarray(wv).astype(BF16),
            "wp": np.ascontiguousarray(wp).astype(BF16),
            "smu_q": (blkdiag * (1.0 / (64.0 * g2q))[:, None]).astype(BF16),
            "ssq_q": (blkdiag * (1.0 / (64.0 * g2q * g2q))[:, None]).astype(BF16),
            "smu_k": (blkdiag * (1.0 / (64.0 * g2k))[:, None]).astype(BF16),
            "ssq_k": (blkdiag * (1.0 / (64.0 * g2k * g2k))[:, None]).astype(BF16),
            "selc": selc.astype(BF16),
            "gq": g2q[:, None].copy(), "gk": g2k[:, None].copy(),
            "dzer": np.zeros((128, 1024), f32).astype(BF16),
        })
    return in_maps


def _numpy_fallback(x, W_qkv, q_gamma, q_beta, k_gamma, k_beta, W_proj, b_proj):
    def ln(t, gamma, beta):
        mu = t.mean(-1, keepdims=True)
        var = ((t - mu) ** 2).mean(-1, keepdims=True)
        return (t - mu) / np.sqrt(var + EPS) * gamma + beta
    Bs, Ns, C = x.shape
    qkv = np.einsum('bnc,oc->bno', x, W_qkv)
    qkv = qkv.reshape(Bs, Ns, 3, HEADS, HD).transpose(2, 0, 3, 1, 4)
    q, k, v = ln(qkv[0], q_gamma, q_beta), ln(qkv[1], k_gamma, k_beta), qkv[2]
    s = np.einsum('bhnd,bhmd->bhnm', q * SCALE, k)
    s = np.exp(s - s.max(-1, keepdims=True))
    p = s / s.sum(-1, keepdims=True)
    o = np.einsum('bhnm,bhmd->bhnd', p, v)
    o = o.transpose(0, 2, 1, 3).reshape(Bs, Ns, C)
    return (np.einsum('bnc,oc->bno', o, W_proj) + b_proj).astype(np.float32)


def kernel(x, W_qkv, q_gamma, q_beta, k_gamma, k_beta, W_proj, b_proj):
    x = np.asarray(x, np.float32)
    W_qkv = np.asarray(W_qkv, np.float32)
    q_gamma = np.asarray(q_gamma, np.float32)
    q_beta = np.asarray(q_beta, np.float32)
    k_gamma = np.asarray(k_gamma, np.float32)
    k_beta = np.asarray(k_beta, np.float32)
    W_proj = np.asarray(W_proj, np.float32)
    b_proj = np.asarray(b_proj, np.float32)

    if np.any(q_beta != 0) or np.any(k_beta != 0):
        # beta terms are not wired into the device kernel (reference always
        # uses beta = 0); fall back to exact host computation
        return _numpy_fallback(x, W_qkv, q_gamma, q_beta, k_gamma, k_beta,
                               W_proj, b_proj)

    from concourse import bass_utils

    if "prog" not in _prog_cache:
        _prog_cache["prog"] = _build_program()
    nc = _prog_cache["prog"]

    in_maps = _prep_core_inputs(x, W_qkv, q_gamma, k_gamma, W_proj)
    res = bass_utils.run_bass_kernel_spmd(nc, in_maps, list(range(N_CORES)))

    out = np.empty((B, N, DIM), np.float32)
    for b in range(B):
        acc = res.results[4 * b + 0]["y"].astype(np.float32).copy()
        for g in range(1, 4):
            acc += res.results[4 * b + g]["y"]
        out[b] = acc + b_proj
    return out
